# revision 1
# baseline (speedup 1.0000x reference)
"""Multi-head attention block (B=4, N=2048, D=768, H=12) on 8 trn2 cores.

Sharding: core c = (batch b = c//2, head-group hg = c%2 of 6 heads).
Each core computes qkv projection for its 6 heads (full sequence), flash-style
attention (scores kept transposed [keys, queries] so softmax normalization can
be fused into the AV matmul via an appended ones-column on V), and a partial
output projection over its heads' 384 hidden dims. The host sums the two
partial projections per batch and adds the bias.

Layout notes (per core):
  xT  [768, 2048]  bf16 : x[b].T           (contraction dim on partitions)
  wT  [768, 1152]  bf16 : [Wq | Wk | Wv] for the 6 heads, transposed
  wpT [384, 768]   bf16 : w_proj[:, f-block].T
  out [2048, 768]  f32  : partial projection (no bias)

Matmul convention: out = lhsT.T @ rhs, contraction on partitions.
  QT/KT [hd, seq]: per 128-row tile = one head *pair* (head dims 0:64 / 64:128)
    -> score matmuls for the two heads use disjoint PE row groups and run
       concurrently (contraction is only 64).
  S^T = K @ Q^T  [keys, queries]; exp via ACT directly from PSUM with the
    1/8 scale folded into the activation's affine pre-scale. No max
    subtraction: logits are ~N(0,1), max ~6, exp is safe in fp32/bf16.
  AV: lhsT = [V_h | ones] (65 cols) accumulates both O'^T and the softmax
    denominator row in one pass. Normalization multiplies the 64 O' rows by
    the broadcast reciprocal (DRAM-bounce partition broadcast).
"""

import json
import sys

import numpy as np

sys.path.insert(0, "/opt/trn_rl_repo")

import ml_dtypes

import concourse.bass as bass
import concourse.bass2jax as bass2jax
import concourse.bass_utils as bass_utils
import concourse.tile as tile
from concourse import mybir

BF16 = ml_dtypes.bfloat16

B, N, D = 4, 2048, 768
H, DH = 12, 64
HG = 2            # head groups (cores per batch)
HL = H // HG      # heads per core
FB = HL * DH      # 384, f-dims per core
KC = D // 128     # 6 contraction chunks
JC = N // 128     # 16 key chunks
SCALE = DH ** -0.5

# ---------------------------------------------------------------------------
# walrus in this container rejects >1 sync wait per instruction; split extra
# waits onto same-engine single-wait Drains inserted just before the owner.
# ---------------------------------------------------------------------------
_orig_compile_bir_kernel = bass_utils.compile_bir_kernel


def _split_multiwaits_json(bir_json: bytes) -> bytes:
    d = json.loads(bir_json)
    n = 0
    changed = False
    for fn in d.get("functions", []):
        for blk in fn.get("blocks", []):
            out = []
            for inst in blk["instructions"]:
                si = inst.get("sync_info") or {}
                waits = si.get("on_wait") or []
                if len(waits) > 1:
                    changed = True
                    for w in waits[:-1]:
                        n += 1
                        out.append({
                            "debug": inst.get("debug"),
                            "engine": inst["engine"],
                            "ins": [],
                            "name": f"I-wsplit-{n}",
                            "opcode": "Drain",
                            "outs": [],
                            "is_reset_sema": False,
                            "sync_info": {"on_update": [], "on_wait": [w]},
                        })
                    si["on_wait"] = [waits[-1]]
                out.append(inst)
            blk["instructions"] = out
    return json.dumps(d).encode() if changed else bir_json


def _patched_compile_bir_kernel(bir_json, tmpdir, neff_name="file.neff"):
    return _orig_compile_bir_kernel(
        _split_multiwaits_json(bir_json), tmpdir, neff_name
    )


bass_utils.compile_bir_kernel = _patched_compile_bir_kernel
bass2jax.compile_bir_kernel = _patched_compile_bir_kernel



# ---------------------------------------------------------------------------
# kernel body
# ---------------------------------------------------------------------------
def build_attention_nc(loop_iters: int | None = None, do_scores=True,
                       do_av=True, do_proj=True) -> bass.Bass:
    """Build the per-core attention program.

    loop_iters: if set, wrap the whole body in a hardware For_i loop that
    re-executes it that many times (timing harness only - amortizes the
    multi-ms axon dispatch overhead; outputs are simply rewritten).
    """
    f32 = mybir.dt.float32
    bf16 = mybir.dt.bfloat16
    nc = bass.Bass()
    xT = nc.declare_dram_parameter("xT", [D, N], bf16, isOutput=False)
    wT = nc.declare_dram_parameter("wT", [D, 3 * FB], bf16, isOutput=False)
    wpT = nc.declare_dram_parameter("wpT", [FB, D], bf16, isOutput=False)
    out = nc.declare_dram_parameter("out", [N, D], f32, isOutput=True)
    rl_dram = nc.dram_tensor("rl_scratch", [HL, N], f32)
    rl2_dram = nc.dram_tensor("rl2_scratch", [HL, N], f32)

    xT_r = xT.rearrange("(c p) i -> c p i", p=128)
    wT_r = wT.rearrange("(c p) m -> c p m", p=128)

    with tile.TileContext(nc) as tc:
        with (
            tc.tile_pool(name="singles", bufs=1) as singles,
            tc.tile_pool(name="epool", bufs=8) as epool,
            tc.tile_pool(name="otpool", bufs=2) as otpool,
            tc.tile_pool(name="rlbpool", bufs=2) as rlbpool,
            tc.tile_pool(name="outpool", bufs=2) as outpool,
            tc.tile_pool(name="pp", bufs=2, space="PSUM") as pp,
            tc.tile_pool(name="avp", bufs=2, space="PSUM") as avp,
        ):
            def body():
                # ---- inputs, loaded per contraction chunk so the first
                # qkv matmuls can start after ~2us instead of ~16us ----
                xT_sb, wq_sb = [], []
                for c in range(KC):
                    xc = singles.tile([128, N], bf16, name=f"xT{c}")
                    nc.sync.dma_start(xc[:], xT_r[c])
                    xT_sb.append(xc)
                    wc = singles.tile([128, 3 * FB], bf16, name=f"wT{c}")
                    nc.sync.dma_start(wc[:], wT_r[c])
                    wq_sb.append(wc)
                wp_sb = singles.tile([64, HL, D], bf16, name="wp")
                nc.sync.dma_start(
                    wp_sb[:], wpT.rearrange("(h p) e -> p h e", p=64)
                )

                # ---- resident intermediates ----
                QT_sb = singles.tile([128, HL // 2, N], bf16, name="QT")
                KT_sb = singles.tile([128, HL // 2, N], bf16, name="KT")
                V_sb = singles.tile([128, JC, HL, DH + 1], bf16, name="V")
                O_norm = singles.tile([64, HL, N], bf16, name="On")
                nc.vector.memset(V_sb[:, :, :, DH:DH + 1], 1.0)

                # ---- PE work-unit emitters (one PSUM group each) ----
                def emit_qk(t, i4pair):
                    # two [head-dim 128, seq 512] tiles of Q^T (t<3) / K^T,
                    # c-outer i4-inner so each stationary w-chunk loads once
                    dst = QT_sb if t < 3 else KT_sb
                    pair = t % 3
                    ps = pp.tile([128, 1024], f32, tag="sc", name="ps_qk")
                    for c in range(KC):
                        for s in range(2):
                            i4 = 2 * i4pair + s
                            nc.tensor.matmul(
                                ps[:, s * 512:(s + 1) * 512],
                                wq_sb[c][:, t * 128:(t + 1) * 128],
                                xT_sb[c][:, i4 * 512:(i4 + 1) * 512],
                                start=(c == 0),
                                stop=(c == KC - 1),
                            )
                    nc.vector.tensor_copy(
                        dst[:, pair, i4pair * 1024:(i4pair + 1) * 1024], ps
                    )

                def emit_v(jc):
                    # one [seq 128, 384] slab of V (plus the ones column)
                    ps = pp.tile([128, 1024], f32, tag="sc", name="ps_v")
                    ps = ps[:, :FB]
                    for c in range(KC):
                        nc.tensor.matmul(
                            ps,
                            xT_sb[c][:, jc * 128:(jc + 1) * 128],
                            wq_sb[c][:, 2 * FB:3 * FB],
                            start=(c == 0),
                            stop=(c == KC - 1),
                        )
                    nc.vector.tensor_copy(
                        V_sb[:, jc, :, 0:DH],
                        ps.rearrange("p (h d) -> p h d", h=HL),
                    )

                def emit_proj(it):
                    # one [seq 128, 768] output tile; h-outer so each O_norm
                    # chunk (stationary) loads once for both e-halves
                    ps = pp.tile([128, 1024], f32, tag="sc", name="ps_pj")
                    for h in range(HL):
                        for eh in range(2):
                            # eh=1 starts at col 512 so each matmul output
                            # stays inside one PSUM bank
                            nc.tensor.matmul(
                                ps[:, eh * 512:eh * 512 + 384],
                                O_norm[:, h, it * 128:(it + 1) * 128],
                                wp_sb[:, h, eh * 384:(eh + 1) * 384],
                                start=(h == 0),
                                stop=(h == HL - 1),
                            )
                    ob = outpool.tile([128, D], f32, tag="ob", name="ob")
                    nc.vector.tensor_copy(ob[:, 0:384], ps[:, 0:384])
                    nc.vector.tensor_copy(ob[:, 384:768], ps[:, 512:896])
                    nc.sync.dma_start(out[it * 128:(it + 1) * 128, :], ob[:])

                # Filler queue: PE work interleaved into the ACT-paced
                # attention loop (engines execute in per-engine program
                # order, so bubbles must be filled at emission time).
                fillers = []
                for t in (1, 4):
                    fillers += [lambda t=t, i=i: emit_qk(t, i) for i in range(2)]
                fillers += [lambda jc=jc: emit_v(jc) for jc in range(10, JC)]
                for t in (2, 5):
                    fillers += [lambda t=t, i=i: emit_qk(t, i) for i in range(2)]

                # ---- upfront qkv: pair-0 Q/K + the first V slabs ----
                for t in (0, 3):
                    for i4pair in range(2):
                        emit_qk(t, i4pair)
                for jc in range(10):
                    emit_v(jc)

                def flash(pair, half):
                    q0 = half * 1024
                    avs = [
                        avp.tile([DH + 1, 1024], f32, tag="av", name=f"av{s}")
                        for s in range(2)
                    ]
                    for jc in range(JC):
                        e_tiles = {}
                        for sub in range(2):      # head = 2*pair + sub
                            r0 = 64 * sub
                            ps = pp.tile([128, 1024], f32, tag="sc",
                                         name="ps_sc")
                            for q in range(2):
                                nc.tensor.matmul(
                                    ps[:, q * 512:(q + 1) * 512],
                                    KT_sb[r0:r0 + 64, pair,
                                          jc * 128:(jc + 1) * 128],
                                    QT_sb[r0:r0 + 64, pair,
                                          q0 + q * 512:q0 + (q + 1) * 512],
                                    start=True,
                                    stop=True,
                                )
                            et = epool.tile([128, 1024], bf16, tag="e")
                            nc.scalar.activation(
                                et[:], ps[:],
                                mybir.ActivationFunctionType.Exp,
                                scale=float(SCALE),
                            )
                            e_tiles[sub] = et
                        if fillers:
                            fillers.pop(0)()
                        if not do_av:
                            continue
                        for sub in range(2):
                            h = 2 * pair + sub
                            for q in range(2):
                                nc.tensor.matmul(
                                    avs[sub][:, q * 512:(q + 1) * 512],
                                    V_sb[:, jc, h, :],
                                    e_tiles[sub][:, q * 512:(q + 1) * 512],
                                    start=(jc == 0),
                                    stop=(jc == JC - 1),
                                )
                    # finalize both heads of the pair for this query half
                    if not do_av:
                        return
                    for sub in range(2):
                        h = 2 * pair + sub
                        ot = otpool.tile([DH + 1, 1024], f32, tag="ot")
                        nc.vector.tensor_copy(ot[0:DH, :], avs[sub][0:DH, :])
                        nc.vector.tensor_copy(
                            ot[DH:DH + 1, :], avs[sub][DH:DH + 1, :]
                        )
                        # reciprocal is ~8 cyc/element and partition-serial;
                        # bounce through DRAM to spread the row over 64
                        # partitions first (8.5us -> 0.15us)
                        nc.sync.dma_start(
                            rl_dram[h:h + 1, q0:q0 + 1024], ot[DH:DH + 1, :]
                        )
                        rs = rlbpool.tile([64, 16], f32, tag="rs")
                        nc.sync.dma_start(
                            rs[:],
                            rl_dram[h, q0:q0 + 1024].rearrange(
                                "(p c) -> p c", p=64),
                        )
                        nc.vector.reciprocal(rs[:], rs[:])
                        nc.sync.dma_start(
                            rl2_dram[h, q0:q0 + 1024].rearrange(
                                "(p c) -> p c", p=64),
                            rs[:],
                        )
                        rlb = rlbpool.tile([64, 1024], f32, tag="rlb")
                        rl_src = rl2_dram[h, q0:q0 + 1024]
                        rl_bcast = bass.AP(
                            tensor=rl_src.tensor,
                            offset=rl_src.offset,
                            ap=[[0, 64]] + list(rl_src.ap),
                        )
                        nc.sync.dma_start(rlb[:], rl_bcast)
                        nc.vector.tensor_mul(
                            O_norm[:, h, q0:q0 + 1024], ot[0:DH, :], rlb[:]
                        )

                # ---- half 0 of all pairs, fillers = remaining qkv ----
                if do_scores:
                    for pair in range(HL // 2):
                        flash(pair, 0)
                # ---- half 1: first-half projection rides in the bubbles ----
                if do_proj:
                    fillers += [lambda it=it: emit_proj(it) for it in range(8)]
                if do_scores:
                    for pair in range(HL // 2):
                        flash(pair, 1)
                while fillers:
                    fillers.pop(0)()
                # ---- tail projection ----
                if do_proj:
                    for it in range(8, 16):
                        emit_proj(it)

            if loop_iters is None:
                body()
            else:
                with tc.For_i(0, loop_iters, 1):
                    body()

    return nc


# ---------------------------------------------------------------------------
# host-side runner (cached jitted executable, per-core input packing)
# ---------------------------------------------------------------------------
_RUNNER = None


def _get_runner():
    global _RUNNER
    if _RUNNER is None:
        nc = build_attention_nc()
        _RUNNER = _make_runner(nc, n_cores=8)
    return _RUNNER


def _make_runner(nc, n_cores):
    """Build the sharded jitted executable once (mirrors run_bass_via_pjrt)."""
    import jax
    from jax.sharding import Mesh, PartitionSpec
    from jax.experimental.shard_map import shard_map

    bass2jax.install_neuronx_cc_hook()

    partition_name = (
        nc.partition_id_tensor.name if nc.partition_id_tensor else None
    )
    in_names, out_names, out_avals, zero_outs = [], [], [], []
    for alloc in nc.m.functions[0].allocations:
        if not isinstance(alloc, mybir.MemoryLocationSet):
            continue
        name = alloc.memorylocations[0].name
        if alloc.kind == "ExternalInput":
            if name != partition_name:
                in_names.append(name)
        elif alloc.kind == "ExternalOutput":
            out_names.append(name)
            shape = tuple(alloc.tensor_shape)
            dtype = mybir.dt.np(alloc.dtype)
            out_avals.append(jax.core.ShapedArray(shape, dtype))
            zero_outs.append(np.zeros(shape, dtype))
    n_params = len(in_names)
    all_in_names = in_names + out_names
    if partition_name is not None:
        all_in_names = all_in_names + [partition_name]

    def _body(*args):
        operands = list(args)
        if partition_name is not None:
            operands.append(bass2jax.partition_id_tensor())
        outs = bass2jax._bass_exec_p.bind(
            *operands,
            out_avals=tuple(out_avals),
            in_names=tuple(all_in_names),
            out_names=tuple(out_names),
            lowering_input_output_aliases=(),
            sim_require_finite=True,
            sim_require_nnan=True,
            nc=nc,
        )
        return tuple(outs)

    devices = jax.devices()[:n_cores]
    mesh = Mesh(np.asarray(devices), ("core",))
    n_outs = len(out_names)
    sharded = jax.jit(
        shard_map(
            _body,
            mesh=mesh,
            in_specs=(PartitionSpec("core"),) * (n_params + n_outs),
            out_specs=(PartitionSpec("core"),) * n_outs,
            check_rep=False,
        ),
        donate_argnums=tuple(range(n_params, n_params + n_outs)),
        keep_unused=True,
    )

    def pack(in_maps):
        concat_in = [
            np.concatenate([np.asarray(m[name]) for m in in_maps], axis=0)
            for name in in_names
        ]
        concat_zero = [
            np.zeros((n_cores * z.shape[0], *z.shape[1:]), z.dtype)
            for z in zero_outs
        ]
        return concat_in, concat_zero

    def unpack(out_arrs):
        return [
            {
                name: np.asarray(out_arrs[i]).reshape(
                    n_cores, *out_avals[i].shape
                )[c]
                for i, name in enumerate(out_names)
            }
            for c in range(n_cores)
        ]

    def run(in_maps):
        concat_in, concat_zero = pack(in_maps)
        return unpack(sharded(*concat_in, *concat_zero))

    run.in_names = in_names
    run.out_names = out_names
    run.pack = pack
    run.unpack = unpack
    run.sharded = sharded
    run.mesh = mesh
    return run


def make_in_maps(x, w_qkv, w_proj):
    """Shard/pack full inputs into the 8 per-core input maps."""
    W = np.ascontiguousarray(w_qkv).reshape(3, H, DH, D)
    in_maps = []
    for c in range(8):
        b, hg = c // HG, c % HG
        hs = slice(hg * HL, (hg + 1) * HL)
        wg = np.concatenate(
            [W[0, hs].reshape(FB, D), W[1, hs].reshape(FB, D),
             W[2, hs].reshape(FB, D)], axis=0)                     # [1152, 768]
        in_maps.append({
            "xT": np.ascontiguousarray(x[b].T).astype(BF16),
            "wT": np.ascontiguousarray(wg.T).astype(BF16),
            "wpT": np.ascontiguousarray(
                w_proj[:, hg * FB:(hg + 1) * FB].T).astype(BF16),
        })
    return in_maps


def kernel(x, w_qkv, w_proj, b_proj):
    x = np.asarray(x, dtype=np.float32)
    w_qkv = np.asarray(w_qkv, dtype=np.float32)
    w_proj = np.asarray(w_proj, dtype=np.float32)
    b_proj = np.asarray(b_proj, dtype=np.float32)

    run = _get_runner()
    results = run(make_in_maps(x, w_qkv, w_proj))

    out = np.empty((B, N, D), dtype=np.float32)
    for b in range(B):
        out[b] = results[2 * b]["out"] + results[2 * b + 1]["out"] + b_proj
    return out



# revision 2
# speedup vs baseline: 22.2007x; 22.2007x over previous
"""Multi-head attention block (B=4, N=2048, D=768, H=12) on 8 trn2 cores.

Sharding: core c = (batch b = c//2, head-group hg = c%2 of 6 heads).
Each core computes qkv projection for its 6 heads (full sequence), flash-style
attention (scores kept transposed [keys, queries] so softmax normalization can
be fused into the AV matmul via an appended ones-column on V), and a partial
output projection over its heads' 384 hidden dims. The host sums the two
partial projections per batch and adds the bias.

Layout notes (per core):
  xT  [768, 2048]  bf16 : x[b].T           (contraction dim on partitions)
  wT  [768, 1152]  bf16 : [Wq | Wk | Wv] for the 6 heads, transposed
  wpT [384, 768]   bf16 : w_proj[:, f-block].T
  out [2048, 768]  f32  : partial projection (no bias)

Matmul convention: out = lhsT.T @ rhs, contraction on partitions.
  QT/KT [hd, seq]: per 128-row tile = one head *pair* (head dims 0:64 / 64:128)
    -> score matmuls for the two heads use disjoint PE row groups and run
       concurrently (contraction is only 64).
  S^T = K @ Q^T  [keys, queries]; exp via ACT directly from PSUM with the
    1/8 scale folded into the activation's affine pre-scale. No max
    subtraction: logits are ~N(0,1), max ~6, exp is safe in fp32/bf16.
  AV: lhsT = [V_h | ones] (65 cols) accumulates both O'^T and the softmax
    denominator row in one pass. Normalization multiplies the 64 O' rows by
    the broadcast reciprocal (DRAM-bounce partition broadcast).
"""

import json
import sys

import numpy as np

sys.path.insert(0, "/opt/trn_rl_repo")

import ml_dtypes

import concourse.bass as bass
import concourse.bass2jax as bass2jax
import concourse.bass_utils as bass_utils
import concourse.tile as tile
from concourse import mybir

BF16 = ml_dtypes.bfloat16

B, N, D = 4, 2048, 768
H, DH = 12, 64
HG = 2            # head groups (cores per batch)
HL = H // HG      # heads per core
FB = HL * DH      # 384, f-dims per core
KC = D // 128     # 6 contraction chunks
JC = N // 128     # 16 key chunks
SCALE = DH ** -0.5

# ---------------------------------------------------------------------------
# walrus in this container rejects >1 sync wait per instruction; split extra
# waits onto same-engine single-wait Drains inserted just before the owner.
# ---------------------------------------------------------------------------
_orig_compile_bir_kernel = bass_utils.compile_bir_kernel


def _split_multiwaits_json(bir_json: bytes) -> bytes:
    d = json.loads(bir_json)
    n = 0
    changed = False
    for fn in d.get("functions", []):
        for blk in fn.get("blocks", []):
            out = []
            for inst in blk["instructions"]:
                si = inst.get("sync_info") or {}
                waits = si.get("on_wait") or []
                if len(waits) > 1:
                    changed = True
                    for w in waits[:-1]:
                        n += 1
                        out.append({
                            "debug": inst.get("debug"),
                            "engine": inst["engine"],
                            "ins": [],
                            "name": f"I-wsplit-{n}",
                            "opcode": "Drain",
                            "outs": [],
                            "is_reset_sema": False,
                            "sync_info": {"on_update": [], "on_wait": [w]},
                        })
                    si["on_wait"] = [waits[-1]]
                out.append(inst)
            blk["instructions"] = out
    return json.dumps(d).encode() if changed else bir_json


def _patched_compile_bir_kernel(bir_json, tmpdir, neff_name="file.neff"):
    return _orig_compile_bir_kernel(
        _split_multiwaits_json(bir_json), tmpdir, neff_name
    )


bass_utils.compile_bir_kernel = _patched_compile_bir_kernel
bass2jax.compile_bir_kernel = _patched_compile_bir_kernel



# ---------------------------------------------------------------------------
# kernel body
# ---------------------------------------------------------------------------
def build_attention_nc(loop_iters: int | None = None, do_scores=True,
                       do_av=True, do_proj=True) -> bass.Bass:
    """Build the per-core attention program.

    loop_iters: if set, wrap the whole body in a hardware For_i loop that
    re-executes it that many times (timing harness only - amortizes the
    multi-ms axon dispatch overhead; outputs are simply rewritten).
    """
    f32 = mybir.dt.float32
    bf16 = mybir.dt.bfloat16
    nc = bass.Bass()
    xT = nc.declare_dram_parameter("xT", [D, N], bf16, isOutput=False)
    wT = nc.declare_dram_parameter("wT", [D, 3 * FB], bf16, isOutput=False)
    wpT = nc.declare_dram_parameter("wpT", [FB, D], bf16, isOutput=False)
    out = nc.declare_dram_parameter("out", [N, D], f32, isOutput=True)
    rl_dram = nc.dram_tensor("rl_scratch", [HL, N], f32)
    rl2_dram = nc.dram_tensor("rl2_scratch", [HL, N], f32)

    xT_r = xT.rearrange("(c p) i -> c p i", p=128)
    wT_r = wT.rearrange("(c p) m -> c p m", p=128)

    with tile.TileContext(nc) as tc:
        with (
            tc.tile_pool(name="singles", bufs=1) as singles,
            tc.tile_pool(name="epool", bufs=8) as epool,
            tc.tile_pool(name="otpool", bufs=2) as otpool,
            tc.tile_pool(name="rlbpool", bufs=2) as rlbpool,
            tc.tile_pool(name="outpool", bufs=2) as outpool,
            tc.tile_pool(name="pp", bufs=2, space="PSUM") as pp,
            tc.tile_pool(name="avp", bufs=2, space="PSUM") as avp,
        ):
            def body():
                # ---- inputs, loaded per contraction chunk so the first
                # qkv matmuls can start after ~2us instead of ~16us ----
                xT_sb, wq_sb = [], []
                for c in range(KC):
                    xc = singles.tile([128, N], bf16, name=f"xT{c}")
                    nc.sync.dma_start(xc[:], xT_r[c])
                    xT_sb.append(xc)
                    wc = singles.tile([128, 3 * FB], bf16, name=f"wT{c}")
                    nc.sync.dma_start(wc[:], wT_r[c])
                    wq_sb.append(wc)
                wp_sb = singles.tile([64, HL, D], bf16, name="wp")
                nc.sync.dma_start(
                    wp_sb[:], wpT.rearrange("(h p) e -> p h e", p=64)
                )

                # ---- resident intermediates ----
                QT_sb = singles.tile([128, HL // 2, N], bf16, name="QT")
                KT_sb = singles.tile([128, HL // 2, N], bf16, name="KT")
                V_sb = singles.tile([128, JC, HL, DH + 1], bf16, name="V")
                O_norm = singles.tile([64, HL, N], bf16, name="On")
                nc.vector.memset(V_sb[:, :, :, DH:DH + 1], 1.0)
                if not do_scores and do_proj:
                    # timing-ablation only: proj reads O_norm, give it data
                    nc.gpsimd.memset(O_norm[:], 0.0)

                # ---- PE work-unit emitters (one PSUM group each) ----
                def emit_qk(t, i4pair):
                    # two [head-dim 128, seq 512] tiles of Q^T (t<3) / K^T,
                    # c-outer i4-inner so each stationary w-chunk loads once
                    dst = QT_sb if t < 3 else KT_sb
                    pair = t % 3
                    ps = pp.tile([128, 1024], f32, tag="sc", name="ps_qk")
                    for c in range(KC):
                        for s in range(2):
                            i4 = 2 * i4pair + s
                            nc.tensor.matmul(
                                ps[:, s * 512:(s + 1) * 512],
                                wq_sb[c][:, t * 128:(t + 1) * 128],
                                xT_sb[c][:, i4 * 512:(i4 + 1) * 512],
                                start=(c == 0),
                                stop=(c == KC - 1),
                            )
                    nc.vector.tensor_copy(
                        dst[:, pair, i4pair * 1024:(i4pair + 1) * 1024], ps
                    )

                def emit_v(jc):
                    # one [seq 128, 384] slab of V (plus the ones column)
                    ps = pp.tile([128, 1024], f32, tag="sc", name="ps_v")
                    ps = ps[:, :FB]
                    for c in range(KC):
                        nc.tensor.matmul(
                            ps,
                            xT_sb[c][:, jc * 128:(jc + 1) * 128],
                            wq_sb[c][:, 2 * FB:3 * FB],
                            start=(c == 0),
                            stop=(c == KC - 1),
                        )
                    nc.vector.tensor_copy(
                        V_sb[:, jc, :, 0:DH],
                        ps.rearrange("p (h d) -> p h d", h=HL),
                    )

                def emit_proj(it):
                    # one [seq 128, 768] output tile; h-outer so each O_norm
                    # chunk (stationary) loads once for both e-halves
                    ps = pp.tile([128, 1024], f32, tag="sc", name="ps_pj")
                    for h in range(HL):
                        for eh in range(2):
                            # eh=1 starts at col 512 so each matmul output
                            # stays inside one PSUM bank
                            nc.tensor.matmul(
                                ps[:, eh * 512:eh * 512 + 384],
                                O_norm[:, h, it * 128:(it + 1) * 128],
                                wp_sb[:, h, eh * 384:(eh + 1) * 384],
                                start=(h == 0),
                                stop=(h == HL - 1),
                            )
                    ob = outpool.tile([128, D], f32, tag="ob", name="ob")
                    nc.vector.tensor_copy(ob[:, 0:384], ps[:, 0:384])
                    nc.vector.tensor_copy(ob[:, 384:768], ps[:, 512:896])
                    nc.sync.dma_start(out[it * 128:(it + 1) * 128, :], ob[:])

                # Filler queue: PE work interleaved into the ACT-paced
                # attention loop (engines execute in per-engine program
                # order, so bubbles must be filled at emission time).
                fillers = []
                for t in (1, 4):
                    fillers += [lambda t=t, i=i: emit_qk(t, i) for i in range(2)]
                fillers += [lambda jc=jc: emit_v(jc) for jc in range(10, JC)]
                for t in (2, 5):
                    fillers += [lambda t=t, i=i: emit_qk(t, i) for i in range(2)]

                # ---- upfront qkv: pair-0 Q/K + the first V slabs ----
                for t in (0, 3):
                    for i4pair in range(2):
                        emit_qk(t, i4pair)
                for jc in range(10):
                    emit_v(jc)

                def flash(pair, half):
                    q0 = half * 1024
                    avs = [
                        avp.tile([DH + 1, 1024], f32, tag="av", name=f"av{s}")
                        for s in range(2)
                    ]
                    for jc in range(JC):
                        e_tiles = {}
                        for sub in range(2):      # head = 2*pair + sub
                            r0 = 64 * sub
                            ps = pp.tile([128, 1024], f32, tag="sc",
                                         name="ps_sc")
                            for q in range(2):
                                nc.tensor.matmul(
                                    ps[:, q * 512:(q + 1) * 512],
                                    KT_sb[r0:r0 + 64, pair,
                                          jc * 128:(jc + 1) * 128],
                                    QT_sb[r0:r0 + 64, pair,
                                          q0 + q * 512:q0 + (q + 1) * 512],
                                    start=True,
                                    stop=True,
                                )
                            et = epool.tile([128, 1024], bf16, tag="e")
                            nc.scalar.activation(
                                et[:], ps[:],
                                mybir.ActivationFunctionType.Exp,
                                scale=float(SCALE),
                            )
                            e_tiles[sub] = et
                        if fillers:
                            fillers.pop(0)()
                        if not do_av:
                            continue
                        for sub in range(2):
                            h = 2 * pair + sub
                            for q in range(2):
                                nc.tensor.matmul(
                                    avs[sub][:, q * 512:(q + 1) * 512],
                                    V_sb[:, jc, h, :],
                                    e_tiles[sub][:, q * 512:(q + 1) * 512],
                                    start=(jc == 0),
                                    stop=(jc == JC - 1),
                                )
                    # finalize both heads of the pair for this query half
                    if not do_av:
                        return
                    for sub in range(2):
                        h = 2 * pair + sub
                        ot = otpool.tile([DH + 1, 1024], f32, tag="ot")
                        nc.vector.tensor_copy(ot[0:DH, :], avs[sub][0:DH, :])
                        nc.vector.tensor_copy(
                            ot[DH:DH + 1, :], avs[sub][DH:DH + 1, :]
                        )
                        # reciprocal is ~8 cyc/element and partition-serial;
                        # bounce through DRAM to spread the row over 64
                        # partitions first (8.5us -> 0.15us)
                        nc.sync.dma_start(
                            rl_dram[h:h + 1, q0:q0 + 1024], ot[DH:DH + 1, :]
                        )
                        rs = rlbpool.tile([64, 16], f32, tag="rs")
                        nc.sync.dma_start(
                            rs[:],
                            rl_dram[h, q0:q0 + 1024].rearrange(
                                "(p c) -> p c", p=64),
                        )
                        nc.vector.reciprocal(rs[:], rs[:])
                        nc.sync.dma_start(
                            rl2_dram[h, q0:q0 + 1024].rearrange(
                                "(p c) -> p c", p=64),
                            rs[:],
                        )
                        rlb = rlbpool.tile([64, 1024], f32, tag="rlb")
                        rl_src = rl2_dram[h, q0:q0 + 1024]
                        rl_bcast = bass.AP(
                            tensor=rl_src.tensor,
                            offset=rl_src.offset,
                            ap=[[0, 64]] + list(rl_src.ap),
                        )
                        nc.sync.dma_start(rlb[:], rl_bcast)
                        nc.vector.tensor_mul(
                            O_norm[:, h, q0:q0 + 1024], ot[0:DH, :], rlb[:]
                        )

                # ---- half 0 of all pairs, fillers = remaining qkv ----
                if do_scores:
                    for pair in range(HL // 2):
                        flash(pair, 0)
                # ---- half 1: first-half projection rides in the bubbles ----
                if do_proj:
                    fillers += [lambda it=it: emit_proj(it) for it in range(8)]
                if do_scores:
                    for pair in range(HL // 2):
                        flash(pair, 1)
                while fillers:
                    fillers.pop(0)()
                # ---- tail projection ----
                if do_proj:
                    for it in range(8, 16):
                        emit_proj(it)

            if loop_iters is None:
                body()
            else:
                with tc.For_i(0, loop_iters, 1):
                    body()

    return nc


# ---------------------------------------------------------------------------
# host-side runner (cached jitted executable, per-core input packing)
# ---------------------------------------------------------------------------
_RUNNER = None


def _get_runner():
    global _RUNNER
    if _RUNNER is None:
        nc = build_attention_nc()
        _RUNNER = _make_runner(nc, n_cores=8)
    return _RUNNER


def _make_runner(nc, n_cores):
    """Build the sharded jitted executable once (mirrors run_bass_via_pjrt)."""
    import jax
    from jax.sharding import Mesh, PartitionSpec
    from jax.experimental.shard_map import shard_map

    bass2jax.install_neuronx_cc_hook()

    partition_name = (
        nc.partition_id_tensor.name if nc.partition_id_tensor else None
    )
    in_names, out_names, out_avals, zero_outs = [], [], [], []
    for alloc in nc.m.functions[0].allocations:
        if not isinstance(alloc, mybir.MemoryLocationSet):
            continue
        name = alloc.memorylocations[0].name
        if alloc.kind == "ExternalInput":
            if name != partition_name:
                in_names.append(name)
        elif alloc.kind == "ExternalOutput":
            out_names.append(name)
            shape = tuple(alloc.tensor_shape)
            dtype = mybir.dt.np(alloc.dtype)
            out_avals.append(jax.core.ShapedArray(shape, dtype))
            zero_outs.append(np.zeros(shape, dtype))
    n_params = len(in_names)
    all_in_names = in_names + out_names
    if partition_name is not None:
        all_in_names = all_in_names + [partition_name]

    def _body(*args):
        operands = list(args)
        if partition_name is not None:
            operands.append(bass2jax.partition_id_tensor())
        outs = bass2jax._bass_exec_p.bind(
            *operands,
            out_avals=tuple(out_avals),
            in_names=tuple(all_in_names),
            out_names=tuple(out_names),
            lowering_input_output_aliases=(),
            sim_require_finite=True,
            sim_require_nnan=True,
            nc=nc,
        )
        return tuple(outs)

    devices = jax.devices()[:n_cores]
    mesh = Mesh(np.asarray(devices), ("core",))
    n_outs = len(out_names)
    sharded = jax.jit(
        shard_map(
            _body,
            mesh=mesh,
            in_specs=(PartitionSpec("core"),) * (n_params + n_outs),
            out_specs=(PartitionSpec("core"),) * n_outs,
            check_rep=False,
        ),
        donate_argnums=tuple(range(n_params, n_params + n_outs)),
        keep_unused=True,
    )

    def pack(in_maps):
        concat_in = [
            np.concatenate([np.asarray(m[name]) for m in in_maps], axis=0)
            for name in in_names
        ]
        concat_zero = [
            np.zeros((n_cores * z.shape[0], *z.shape[1:]), z.dtype)
            for z in zero_outs
        ]
        return concat_in, concat_zero

    def unpack(out_arrs):
        return [
            {
                name: np.asarray(out_arrs[i]).reshape(
                    n_cores, *out_avals[i].shape
                )[c]
                for i, name in enumerate(out_names)
            }
            for c in range(n_cores)
        ]

    def run(in_maps):
        concat_in, concat_zero = pack(in_maps)
        return unpack(sharded(*concat_in, *concat_zero))

    run.in_names = in_names
    run.out_names = out_names
    run.pack = pack
    run.unpack = unpack
    run.sharded = sharded
    run.mesh = mesh
    return run


def make_in_maps(x, w_qkv, w_proj):
    """Shard/pack full inputs into the 8 per-core input maps."""
    W = np.ascontiguousarray(w_qkv).reshape(3, H, DH, D)
    in_maps = []
    for c in range(8):
        b, hg = c // HG, c % HG
        hs = slice(hg * HL, (hg + 1) * HL)
        wg = np.concatenate(
            [W[0, hs].reshape(FB, D), W[1, hs].reshape(FB, D),
             W[2, hs].reshape(FB, D)], axis=0)                     # [1152, 768]
        in_maps.append({
            "xT": np.ascontiguousarray(x[b].T).astype(BF16),
            "wT": np.ascontiguousarray(wg.T).astype(BF16),
            "wpT": np.ascontiguousarray(
                w_proj[:, hg * FB:(hg + 1) * FB].T).astype(BF16),
        })
    return in_maps


def kernel(x, w_qkv, w_proj, b_proj):
    x = np.asarray(x, dtype=np.float32)
    w_qkv = np.asarray(w_qkv, dtype=np.float32)
    w_proj = np.asarray(w_proj, dtype=np.float32)
    b_proj = np.asarray(b_proj, dtype=np.float32)

    run = _get_runner()
    results = run(make_in_maps(x, w_qkv, w_proj))

    out = np.empty((B, N, D), dtype=np.float32)
    for b in range(B):
        out[b] = results[2 * b]["out"] + results[2 * b + 1]["out"] + b_proj
    return out

